# revision 5
# baseline (speedup 1.0000x reference)
"""Trainium2 Bass kernel for nn_Memory_efficient_network.

kernel(**inputs) takes the FULL unsharded inputs (as from setup_inputs())
and returns the full output tuple matching reference().

8-core strategy:
  shard = (batch b, trailing-2 k, half h); h halves C for gxc, P for pxg.
  L0 pools are host-precomputed; the 3x-channel concat is shipped
  pre-concatenated (48 ch) so L0 is one K=48 matmul per tile.
  L1 broadcast-pool terms are injected into PSUM by extra matmuls with
  stride-0 (broadcast) moving-operand APs.  lrelu+bias fused into ScalarE
  activation(Lrelu) at PSUM evacuation.  The only cross-core traffic is a
  pairwise AllGather of the two L1 pool partials.  Device emits only
  pooled_gxc / pooled_pxg; heads and tiny branches finish on host (fp32).
Device math: fp16 operands, fp32 PSUM accumulation.
"""

import numpy as np

NEG = 0.01
B, CH, NF, G, C, P = 2, 16, 64, 256, 512, 128
CL = C // 2
PL = P // 2
NCORES = 8
NQ_G = 64          # gxc banks (4 g each)
NQ_P = 16          # pxg banks (4 p each)

_cache = {}


# ===================================================================
# device program
# ===================================================================
def _build_runner():
    import jax
    from jax.sharding import Mesh, PartitionSpec
    from jax.experimental.shard_map import shard_map
    import concourse.bacc as bacc
    import concourse.mybir as mybir
    from concourse.tile import TileContext
    from concourse.bass2jax import (_bass_exec_p, install_neuronx_cc_hook,
                                    partition_id_tensor)

    dt = mybir.dt
    Alu = mybir.AluOpType
    Act = mybir.ActivationFunctionType

    nc = bacc.Bacc("TRN2", target_bir_lowering=False, debug=False,
                   num_devices=NCORES)

    xg_d = nc.dram_tensor("xg", [48, G * CL], dt.float16, kind="ExternalInput")
    xp_d = nc.dram_tensor("xp", [48, PL * G], dt.float16, kind="ExternalInput")
    wg0_d = nc.dram_tensor("wg0", [112, 64], dt.float16, kind="ExternalInput")
    wp0_d = nc.dram_tensor("wp0", [112, 64], dt.float16, kind="ExternalInput")
    w1_names = ["w1gx_x", "w1gx_g", "w1gx_c", "w1px_x", "w1px_p", "w1px_g"]
    w1_d = {n: nc.dram_tensor(n, [128, 64], dt.float16, kind="ExternalInput")
            for n in w1_names}
    bv_d = nc.dram_tensor("bv", [128, 4], dt.float32, kind="ExternalInput")
    pg_d = nc.dram_tensor("pg", [64, CL], dt.float32, kind="ExternalOutput")
    pp_d = nc.dram_tensor("pp", [128, NQ_P * 2], dt.float32,
                          kind="ExternalOutput")
    cc_in = nc.dram_tensor("cc_in", [128, 384], dt.float16)
    cc_out = nc.dram_tensor("cc_out", [256, 384], dt.float16)
    groups = [[0, 1], [2, 3], [4, 5], [6, 7]]

    with TileContext(nc) as tc:
        with tc.tile_pool(name="wpool", bufs=1) as wp, \
             tc.tile_pool(name="big", bufs=1) as bigp, \
             tc.tile_pool(name="chunk", bufs=3) as chp, \
             tc.tile_pool(name="evac", bufs=3) as evp, \
             tc.tile_pool(name="tree", bufs=2) as trp, \
             tc.tile_pool(name="small", bufs=1) as smp, \
             tc.tile_pool(name="ps", bufs=2, space="PSUM") as psp:

            wg0 = wp.tile([112, 64], dt.float16, tag="wg0")
            wp0 = wp.tile([112, 64], dt.float16, tag="wp0")
            nc.sync.dma_start(wg0[:], wg0_d.ap())
            nc.sync.dma_start(wp0[:], wp0_d.ap())
            w1 = {}
            for n in w1_names:
                w1[n] = wp.tile([128, 64], dt.float16, tag=n, name=n)
                nc.sync.dma_start(w1[n][:], w1_d[n].ap())
            bv = wp.tile([128, 4], dt.float32, tag="bv")
            nc.sync.dma_start(bv[:], bv_d.ap())

            X1 = bigp.tile([128, NQ_G * 512], dt.float16, tag="X1")
            Y1 = bigp.tile([128, NQ_P * 512], dt.float16, tag="Y1")

            # ---------------- layer 0 ----------------
            def layer0(src_d, w0, Xbuf, nq, bias_col):
                nchunk = nq // 4
                src = src_d.ap().rearrange("c (m u v s) -> c m v u s",
                                           m=nchunk, u=2, v=2)
                for m in range(nchunk):
                    t = chp.tile([112, 2048], dt.float16, tag="l0chunk")
                    tv = t[:].rearrange("p (u s) -> p u s", u=2)
                    nc.sync.dma_start(tv[0:48], src[:, m, 0])
                    nc.sync.dma_start(tv[64:112], src[:, m, 1])
                    ps = psp.tile([128, 2048], dt.float32, tag="ps")
                    for qq in range(4):
                        v, u = qq % 2, qq // 2
                        R = 64 * v
                        off = qq * 512
                        for half in range(2):
                            nc.tensor.matmul(
                                ps[64 * half:64 * half + 64, off:off + 512],
                                w0[R:R + 48, :],
                                t[R:R + 48,
                                  1024 * u + 512 * half:1024 * u + 512 * half + 512],
                                start=True, stop=True,
                                tile_position=(R, 64 * half))
                    nc.scalar.activation(
                        Xbuf[:, m * 2048:(m + 1) * 2048], ps[:],
                        Act.Lrelu, bias=bv[:, bias_col:bias_col + 1],
                        alpha=NEG)

            layer0(xg_d, wg0, X1, NQ_G, 0)
            layer0(xp_d, wp0, Y1, NQ_P, 1)

            # ---------------- pools of X1 / Y1 ----------------
            def inner_fold(src_ap, nq):
                """max over innermost 256 -> [128, nq, 2, 1] fold layout."""
                cur = src_ap.rearrange("p (q g s) -> p q g s", q=nq, g=2)
                w = 256
                while w > 1:
                    hw = w // 2
                    o = trp.tile([128, nq * 2 * hw], dt.float16, tag="tree")
                    ov = o[:].rearrange("p (q g s) -> p q g s", q=nq, g=2)
                    nc.vector.tensor_tensor(out=ov, in0=cur[:, :, :, 0:hw],
                                            in1=cur[:, :, :, hw:w], op=Alu.max)
                    cur, w = ov, hw
                return cur

            def bank_tree(src_ap, nq):
                """max over banks and glo -> [128, 256] (per-partition-half)"""
                cur = src_ap
                n = nq
                while n > 1:
                    v = cur.rearrange("p (a two s) -> p a two s", two=2, s=512)
                    o = trp.tile([128, (n // 2) * 512], dt.float16, tag="tree")
                    ov = o[:].rearrange("p (a s) -> p a s", s=512)
                    nc.vector.tensor_tensor(out=ov, in0=v[:, :, 0],
                                            in1=v[:, :, 1], op=Alu.max)
                    cur, n = o[:], n // 2
                v = cur.rearrange("p (g s) -> p g s", g=2)
                o = trp.tile([128, 256], dt.float16, tag="tree2")
                nc.vector.tensor_tensor(out=o[:], in0=v[:, 0], in1=v[:, 1],
                                        op=Alu.max)
                return o

            def half_combine(h128, tagbase, dup):
                """[128,256] half-partials -> max of halves in rows 0-63
                (+ optional dup to rows 64-127). Returns [128,256] tile."""
                s = smp.tile([64, 256], dt.float16, tag=tagbase + "s")
                nc.sync.dma_start(s[:], h128[64:128])
                d = smp.tile([128, 256], dt.float16, tag=tagbase + "d")
                nc.vector.tensor_tensor(out=d[0:64], in0=h128[0:64],
                                        in1=s[:], op=Alu.max)
                if dup:
                    nc.sync.dma_start(d[64:128], d[0:64])
                return d

            g1t = inner_fold(X1[:], NQ_G)             # [128,(64,2,1)]
            g1f = smp.tile([128, 128], dt.float16, tag="g1f")
            nc.vector.tensor_copy(g1f[:].rearrange("p (q g) -> p q g", g=2)
                                  .unsqueeze(3), g1t)
            c1h = bank_tree(X1[:], NQ_G)
            c1d = half_combine(c1h[:], "c1", dup=True)

            p1t = inner_fold(Y1[:], NQ_P)             # [128,(16,2,1)]
            p1f = smp.tile([128, 32], dt.float16, tag="p1f")
            nc.vector.tensor_copy(p1f[:].rearrange("p (q g) -> p q g", g=2)
                                  .unsqueeze(3), p1t)
            g2h = bank_tree(Y1[:], NQ_P)
            g2d = half_combine(g2h[:], "g2", dup=True)

            # ---------------- cross-core exchange ----------------
            pay = smp.tile([128, 384], dt.float16, tag="pay")
            nc.vector.tensor_copy(pay[:, 0:128], g1f[:])
            nc.vector.tensor_copy(pay[:, 128:384], g2d[:])
            nc.sync.dma_start(cc_in.ap(), pay[:])
            nc.gpsimd.collective_compute(
                "AllGather", Alu.bypass, replica_groups=groups,
                ins=[cc_in.ap()], outs=[cc_out.ap()])
            blk0 = smp.tile([128, 384], dt.float16, tag="blk0")
            blk1 = smp.tile([128, 384], dt.float16, tag="blk1")
            nc.sync.dma_start(blk0[:], cc_out.ap()[0:128])
            nc.sync.dma_start(blk1[:], cc_out.ap()[128:256])
            mrg = smp.tile([128, 384], dt.float16, tag="mrg")
            nc.vector.tensor_tensor(out=mrg[:], in0=blk0[:], in1=blk1[:],
                                    op=Alu.max)
            nc.vector.tensor_tensor(out=mrg[:], in0=mrg[:], in1=pay[:],
                                    op=Alu.max)
            g1F = mrg[:, 0:128].rearrange("p (q g) -> p q g", g=2)
            g2F = mrg[:, 128:384]
            p1F = p1f[:].rearrange("p (q g) -> p q g", g=2)

            # ---------------- layer 1 ----------------
            def layer1(Xbuf, nq, wx, wa, wb, a_rhs, b_rhs, bias_col, pool_fn):
                Xv = Xbuf[:].rearrange("p (q s) -> p q s", q=nq)
                for m in range(nq // 4):
                    ps = psp.tile([128, 2048], dt.float32, tag="ps")
                    for qq in range(4):
                        q = 4 * m + qq
                        off = qq * 512
                        for half in range(2):
                            R = 64 * half
                            out = ps[R:R + 64, off:off + 512]
                            o3 = out.rearrange("p (g s) -> p g s", g=2)
                            nc.tensor.matmul(out, wx[R:R + 64, :],
                                             Xv[R:R + 64, q],
                                             start=True, stop=False,
                                             tile_position=(R, R))
                            nc.tensor.matmul(o3, wa[R:R + 64, :],
                                             a_rhs(R, q),
                                             start=False, stop=False,
                                             tile_position=(R, R))
                            nc.tensor.matmul(o3, wb[R:R + 64, :],
                                             b_rhs(R, q),
                                             start=False, stop=True,
                                             tile_position=(R, R))
                    ev = evp.tile([128, 2048], dt.float16, tag="evac")
                    nc.scalar.activation(ev[:], ps[:], Act.Lrelu,
                                         bias=bv[:, bias_col:bias_col + 1],
                                         alpha=NEG)
                    pool_fn(m, ev)

            # gxc pooling: max over g == max over banks+glo, accumulate
            accg = smp.tile([128, 256], dt.float16, tag="accg")
            def pool_gxc(m, ev):
                e4 = ev[:].rearrange("p (a g s) -> p a g s", a=4, g=2)
                t1 = evp.tile([128, 1024], dt.float16, tag="pt1")
                t1v = t1[:].rearrange("p (a g s) -> p a g s", a=2, g=2)
                nc.vector.tensor_tensor(out=t1v, in0=e4[:, 0:2],
                                        in1=e4[:, 2:4], op=Alu.max)
                t2 = evp.tile([128, 512], dt.float16, tag="pt2")
                t2v = t2[:].rearrange("p (g s) -> p g s", g=2)
                nc.vector.tensor_tensor(out=t2v, in0=t1v[:, 0],
                                        in1=t1v[:, 1], op=Alu.max)
                if m == 0:
                    nc.vector.tensor_tensor(out=accg[:], in0=t2v[:, 0],
                                            in1=t2v[:, 1], op=Alu.max)
                else:
                    t3 = evp.tile([128, 256], dt.float16, tag="pt3")
                    nc.vector.tensor_tensor(out=t3[:], in0=t2v[:, 0],
                                            in1=t2v[:, 1], op=Alu.max)
                    nc.vector.tensor_tensor(out=accg[:], in0=accg[:],
                                            in1=t3[:], op=Alu.max)

            layer1(X1, NQ_G, w1["w1gx_x"], w1["w1gx_g"], w1["w1gx_c"],
                   lambda R, q: g1F[R:R + 64, q].unsqueeze(2)
                                   .broadcast_to([64, 2, 256]),
                   lambda R, q: c1d[R:R + 64, :].unsqueeze(1)
                                   .broadcast_to([64, 2, 256]),
                   2, pool_gxc)

            # pxg pooling: max over g == inner fold of each evac tile
            ppf = smp.tile([128, NQ_P * 2], dt.float32, tag="ppf")
            def pool_pxg(m, ev):
                cur = ev[:].rearrange("p (q g s) -> p q g s", q=4, g=2)
                w = 256
                while w > 1:
                    hw = w // 2
                    o = evp.tile([128, 4 * 2 * hw], dt.float16, tag="pxt")
                    ov = o[:].rearrange("p (q g s) -> p q g s", q=4, g=2)
                    nc.vector.tensor_tensor(out=ov, in0=cur[:, :, :, 0:hw],
                                            in1=cur[:, :, :, hw:w],
                                            op=Alu.max)
                    cur, w = ov, hw
                nc.vector.tensor_copy(
                    ppf[:, m * 8:(m + 1) * 8]
                    .rearrange("p (q g) -> p q g", g=2).unsqueeze(3), cur)

            layer1(Y1, NQ_P, w1["w1px_x"], w1["w1px_p"], w1["w1px_g"],
                   lambda R, q: p1F[R:R + 64, q].unsqueeze(2)
                                   .broadcast_to([64, 2, 256]),
                   lambda R, q: g2F[R:R + 64, :].unsqueeze(1)
                                   .broadcast_to([64, 2, 256]),
                   3, pool_pxg)

            # pooled_gxc out: cross-half combine -> [64,256] fp32
            pgs = smp.tile([64, 256], dt.float16, tag="pgs")
            nc.sync.dma_start(pgs[:], accg[64:128])
            pgf = smp.tile([64, 256], dt.float32, tag="pgf")
            nc.vector.tensor_tensor(out=pgf[:], in0=accg[0:64], in1=pgs[:],
                                    op=Alu.max)
            nc.sync.dma_start(pg_d.ap(), pgf[:])
            nc.sync.dma_start(pp_d.ap(), ppf[:])

    nc.compile()
    nc.finalize()

    # ------------- hoisted-jit runner -------------
    install_neuronx_cc_hook()
    import concourse.mybir as mybir_m
    partition_name = (nc.partition_id_tensor.name
                      if nc.partition_id_tensor else None)
    in_names, out_names, out_avals, zero_outs = [], [], [], []
    for alloc in nc.m.functions[0].allocations:
        if not isinstance(alloc, mybir_m.MemoryLocationSet):
            continue
        name = alloc.memorylocations[0].name
        if alloc.kind == "ExternalInput":
            if name != partition_name:
                in_names.append(name)
        elif alloc.kind == "ExternalOutput":
            out_names.append(name)
            shape = tuple(alloc.tensor_shape)
            dtp = mybir_m.dt.np(alloc.dtype)
            out_avals.append(jax.core.ShapedArray(shape, dtp))
            zero_outs.append(np.zeros(shape, dtp))
    n_params, n_outs = len(in_names), len(out_avals)
    all_in_names = in_names + out_names + (
        [partition_name] if partition_name else [])

    def _body(*args):
        operands = list(args)
        if partition_name:
            operands.append(partition_id_tensor())
        outs = _bass_exec_p.bind(
            *operands, out_avals=tuple(out_avals),
            in_names=tuple(all_in_names), out_names=tuple(out_names),
            lowering_input_output_aliases=(), sim_require_finite=True,
            sim_require_nnan=True, nc=nc)
        return tuple(outs)

    devices = jax.devices()[:NCORES]
    mesh = Mesh(np.asarray(devices), ("core",))
    in_specs = (PartitionSpec("core"),) * (n_params + n_outs)
    out_specs = (PartitionSpec("core"),) * n_outs
    fn = jax.jit(shard_map(_body, mesh=mesh, in_specs=in_specs,
                           out_specs=out_specs, check_rep=False),
                 keep_unused=True)

    def run(in_maps):
        concat_in = [np.concatenate([in_maps[c][nm] for c in range(NCORES)],
                                    axis=0) for nm in in_names]
        concat_zeros = [np.zeros((NCORES * z.shape[0], *z.shape[1:]), z.dtype)
                        for z in zero_outs]
        out_arrs = fn(*concat_in, *concat_zeros)
        return [
            {name: np.asarray(out_arrs[i]).reshape(NCORES,
                                                   *out_avals[i].shape)[c]
             for i, name in enumerate(out_names)}
            for c in range(NCORES)]

    return run


# ===================================================================
# host side
# ===================================================================
def _lrelu(x):
    return np.maximum(x, NEG * x)


def _pconv(x, w, b):
    return (np.einsum('oc,bc...->bo...', w, x, optimize=True)
            + b.reshape((1, -1) + (1,) * (x.ndim - 2)))


def _prep_inputs(input_GxCx2, input_PxGx2, params):
    """Build the 8 per-core input maps."""
    f16 = np.float16
    p = params
    # static (same on all cores) weights
    w0g = np.ascontiguousarray(p['w_gxc0'].T).astype(f16)        # [48,64]
    w0p = np.ascontiguousarray(p['w_pxg0'].T).astype(f16)
    wg0 = np.zeros((112, 64), f16); wg0[0:48] = w0g; wg0[64:112] = w0g
    wp0 = np.zeros((112, 64), f16); wp0[0:48] = w0p; wp0[64:112] = w0p

    def dupT(w):   # [64,64] -> transposed, duplicated [128,64]
        t = np.ascontiguousarray(w.T).astype(f16)
        return np.concatenate([t, t], axis=0)
    w1 = {
        "w1gx_x": dupT(p['w_gxc1'][:, 0:64]),
        "w1gx_g": dupT(p['w_gxc1'][:, 64:128]),
        "w1gx_c": dupT(p['w_gxc1'][:, 128:192]),
        "w1px_x": dupT(p['w_pxg1'][:, 0:64]),
        "w1px_p": dupT(p['w_pxg1'][:, 64:128]),
        "w1px_g": dupT(p['w_pxg1'][:, 128:192]),
    }
    bvv = np.zeros((128, 4), np.float32)
    for i, n in enumerate(['b_gxc0', 'b_pxg0', 'b_gxc1', 'b_pxg1']):
        bvv[0:64, i] = p[n]; bvv[64:128, i] = p[n]

    # bank reordering: g index g = 4q+2*half+glo; even banks in rows 0-47,
    # odd banks in rows 64-111 of the chunk tile -> DRAM layout
    # [c, (m, u, v, s)] with bank = 4m + 2u + v, s = 1024 (4 halves*256... )
    # Actually bank q spatial = outer-idx {4q..4q+3} x inner(256):
    #   per bank: [(half,glo) 4, 256] where outer = 4q + 2*half + glo.
    def make_aug(x, pool_a, pool_b, nq):
        # x: [16, O, I]; pool_a [16, O] (varies w/ outer), pool_b [16, I]
        ch, O, I = x.shape
        aug = np.empty((48, O, I), np.float32)
        aug[0:16] = x
        aug[16:32] = pool_a[:, :, None]
        aug[32:48] = pool_b[:, None, :]
        # DRAM layout (m,u,v) == natural bank order; within-bank outer
        # order (half,glo) == natural outer order. No reordering needed.
        return np.ascontiguousarray(aug).astype(f16).reshape(48, O * I)

    maps = []
    xg_f = np.asarray(input_GxCx2, np.float32)
    xp_f = np.asarray(input_PxGx2, np.float32)
    for b in range(B):
        for k in range(2):
            xg = xg_f[b, :, :, :, k]                 # [16, G, C]
            g1_0 = xg.max(axis=2)                    # [16, G]
            c1_0 = xg.max(axis=1)                    # [16, C]
            xp = xp_f[b, :, :, :, k]                 # [16, P, G]
            p1_0 = xp.max(axis=2)                    # [16, P]
            g2_0 = xp.max(axis=1)                    # [16, G]
            for h in range(2):
                xg_s = xg[:, :, h * CL:(h + 1) * CL]     # [16, G, CL]
                c1_s = c1_0[:, h * CL:(h + 1) * CL]
                xp_s = xp[:, h * PL:(h + 1) * PL, :]     # [16, PL, G]
                p1_s = p1_0[:, h * PL:(h + 1) * PL]
                m = {
                    "xg": make_aug(xg_s, g1_0, c1_s, NQ_G),
                    "xp": make_aug(xp_s, p1_s, g2_0, NQ_P),
                    "wg0": wg0, "wp0": wp0, "bv": bvv,
                }
                m.update(w1)
                maps.append(m)
    return maps


def _decode_pool_layout(vec128, nq):
    """[128, nq*2] fold-layout -> [64, nq*4] ordered by outer index."""
    v = vec128.reshape(2, 64, nq, 2)        # (half, ch, q, glo)
    out = np.empty((64, nq * 4), vec128.dtype)
    idx = np.arange(nq)[:, None] * 4 + np.arange(2)[None, :]
    out[:, (idx).ravel()] = v[0].reshape(64, nq * 2)
    out[:, (idx + 2).ravel()] = v[1].reshape(64, nq * 2)
    return out


def kernel(input_GxCx2, input_PxGx2, input_P, input_G, input_1, params):
    if "run" not in _cache:
        _cache["run"] = _build_runner()
    run = _cache["run"]

    maps = _prep_inputs(input_GxCx2, input_PxGx2, params)
    results = run(maps)

    # assemble pooled tensors
    pooled_gxc = np.empty((B, 64, C, 2), np.float32)
    pooled_pxg = np.empty((B, 64, P, 2), np.float32)
    ci = 0
    for b in range(B):
        for k in range(2):
            for h in range(2):
                r = results[ci]; ci += 1
                pooled_gxc[b, :, h * CL:(h + 1) * CL, k] = r["pg"]
                pooled_pxg[b, :, h * PL:(h + 1) * PL, k] = \
                    _decode_pool_layout(r["pp"], NQ_P)

    # host-side tiny branches + heads (fp32)
    p = {k2: np.asarray(v, np.float32) for k2, v in params.items()}
    pp_ = np.asarray(input_P, np.float32)
    gg = np.asarray(input_G, np.float32)
    one = None
    for i in range(2):
        pp_ = _lrelu(_pconv(pp_, p[f'w_p{i}'], p[f'b_p{i}']))
        gg = _lrelu(_pconv(gg, p[f'w_g{i}'], p[f'b_g{i}']))
        one = _lrelu(_pconv(gg, p[f'w_1{i}'], p[f'b_1{i}']))

    def bc4(a, b_):
        a2 = np.broadcast_to(a.max(2, keepdims=True), a.shape)
        b2 = np.broadcast_to(b_.max(2, keepdims=True), b_.shape)
        return (np.concatenate([a, a2], 1), np.concatenate([b_, b2], 1))

    a_gxc, a_pxg = bc4(pooled_gxc, pooled_pxg)
    out_a_gxc = _pconv(a_gxc, p['w_act_gxc'], p['b_act_gxc'])
    out_a_pxg = _pconv(a_pxg, p['w_act_pxg'], p['b_act_pxg'])
    out_a_p = _pconv(pp_, p['w_act_p'], p['b_act_p'])
    out_a_g = _pconv(gg, p['w_act_g'], p['b_act_g'])
    out_a_1 = _pconv(one, p['w_act_1'], p['b_act_1'])
    v_gxc, v_pxg = bc4(pooled_gxc, pooled_pxg)
    v1 = _pconv(v_gxc, p['w_cri_gxc'], p['b_cri_gxc'])
    v2 = _pconv(v_pxg, p['w_cri_pxg'], p['b_cri_pxg'])
    v3 = _pconv(pp_, p['w_cri_p'], p['b_cri_p'])
    v4 = _pconv(gg, p['w_cri_g'], p['b_cri_g'])
    v5 = _pconv(one, p['w_cri_1'], p['b_cri_1'])
    value = np.array([v1.mean(3).sum() + v2.mean(3).sum() + v3.sum()
                      + v4.sum() + v5.sum()], np.float32)
    return (out_a_gxc.astype(np.float32), out_a_pxg.astype(np.float32),
            out_a_p.astype(np.float32), out_a_g.astype(np.float32),
            out_a_1.astype(np.float32), value)
